# revision 17
# baseline (speedup 1.0000x reference)
"""AxialAttention (width=False, no positional) as a Bass/Tile kernel on 8 TRN2 NeuronCores.

Sharding: data-parallel over N (8 images -> 8 cores); conv/BN params replicated.

Math (per image, per w-column, per group g of 8; H=128, gp=16):
    qkv = BN1d(w_qkv @ x)            -> q (8ch), k (8ch), v (16ch) per group
    sim = softmax_j(BN(q.k))         (sim BN: additive part cancels in softmax;
                                      multiplicative part folded into k's weights)
    out = BN(sim @ v)                (output BN folded into v's weights/bias;
                                      softmax denominator divided at the end)

Device-side formulation per (w, g):
    qkT[j,i]  = sum_c k[c,j] q[c,i]          (TensorE, K=64 zero-padded window)
    eT[j,i]   = exp(qkT)                      (ScalarE, PSUM->SBUF bf16)
    usv[i,c], s[i] = eT.T @ [v | 1]           (TensorE; ones col gives softmax denom)
    y[op,h=i,w] = usv[i,c] / s[i]             (VectorE reciprocal + per-partition mult)

Layouts (per core):
    xp   [c, (w,h)]       host-pretransposed input, bf16
    qkp  [o_qk, (w,h)]    o_qk = g*16 + (q:0..8 | k:8..16); k rows pre-scaled by sim BN
    B_s  [o, (w,h)]       k_g copied to q_g's partition rows for g=s(mod 4), zeros
                          elsewhere (SBUF->SBUF partition-shift DMAs); stationary side
                          of the K=64 window matmul at partition base 64*(g//4)
    vtp  [j, (w, g, 16+1)] BN-folded v channels + interleaved ones columns, bf16
    acc  [h, (op, w)]     final output accumulator, fp32
"""

import numpy as np
import ml_dtypes

EPS = 1e-5
G = 8
C = 128
H = 128
W = 128
NCORES = 8
NQ = 4            # number of w-passes
WQ = W // NQ      # w-columns per pass
OP = 128          # out_planes

_cached = None


def _build_bass():
    from contextlib import ExitStack

    import concourse.bass as bass
    from concourse import bacc
    import concourse.tile as tile
    import concourse.mybir as mybir
    from concourse.bass import ts, ds

    f32, bf16 = mybir.dt.float32, mybir.dt.bfloat16
    fp16 = mybir.dt.float16
    Exp = mybir.ActivationFunctionType.Exp

    nc = bacc.Bacc("TRN2", target_bir_lowering=False, debug=False)
    xp_d = nc.dram_tensor("xp", [C, W * H], fp16, kind="ExternalInput")
    wqkT_d = nc.dram_tensor("wqkT", [C, 128], fp16, kind="ExternalInput")
    wvT_d = nc.dram_tensor("wvT", [C, 128], fp16, kind="ExternalInput")
    bqk_d = nc.dram_tensor("bqk", [128, 1], f32, kind="ExternalInput")
    bvbc_d = nc.dram_tensor("bvbc", [128, 128], f32, kind="ExternalInput")
    y_d = nc.dram_tensor("y", [NQ, H, OP, WQ], f32, kind="ExternalOutput")

    QCOLS = WQ * H           # free columns per pass in (w,h) layout

    with tile.TileContext(nc) as tc, ExitStack() as ctx:
        const = ctx.enter_context(tc.tile_pool(name="const", bufs=1))
        wqkT = const.tile([C, 128], fp16)
        wvT = const.tile([C, 128], fp16)
        bqk = const.tile([128, 1], f32)
        bvbc = const.tile([128, 128], f32)
        nc.sync.dma_start(wqkT[:], wqkT_d.ap())
        nc.sync.dma_start(wvT[:], wvT_d.ap())
        nc.sync.dma_start(bqk[:], bqk_d.ap())
        nc.sync.dma_start(bvbc[:], bvbc_d.ap())

        apool = ctx.enter_context(tc.tile_pool(name="apool", bufs=2))
        xpool = ctx.enter_context(tc.tile_pool(name="xpool", bufs=2))
        hpool = ctx.enter_context(tc.tile_pool(name="hpool", bufs=2))
        epool = ctx.enter_context(tc.tile_pool(name="epool", bufs=4))
        rpool = ctx.enter_context(tc.tile_pool(name="rpool", bufs=4))
        psmall = ctx.enter_context(tc.tile_pool(name="psmall", bufs=2, space="PSUM"))
        pqk = ctx.enter_context(tc.tile_pool(name="pqk", bufs=3, space="PSUM"))

        NP = WQ // 2

        def build(q):
            """Emit the load/projection/pair-build stage for pass q."""
            xq = xpool.tile([C, QCOLS], fp16, tag="xq")
            qtr = QCOLS // 4
            for i in range(4):
                nc.sync.dma_start(xq[:, ds(i * qtr, qtr)],
                                  xp_d.ap()[:, ds(q * QCOLS + i * qtr, qtr)])

            # q/k projection: qkp[o,(w,h)] = wqkT.T @ xq + bqk (k rows sim-scaled)
            qkp = hpool.tile([128, QCOLS], fp16, tag="qkp")
            for i in range(QCOLS // 512):
                pt = pqk.tile([128, 512], f32, tag="pq")
                nc.tensor.matmul(pt[:], wqkT[:], xq[:, ts(i, 512)],
                                 start=True, stop=True)
                nc.vector.tensor_scalar_add(qkp[:, ts(i, 512)], pt[:], bqk[:, 0:1])

            # vT projection: vtp[j=h, (w, g, c|1)] = xq_w.T @ wvT + bv (BN-folded)
            vtp = hpool.tile([128, WQ * G * 17], bf16, tag="vtp")
            vtp4 = vtp.rearrange("p (w g c) -> p w g c", w=WQ, g=G)
            nc.vector.memset(vtp4[:, :, :, 16:17], 1.0)
            bvbc3 = bvbc.rearrange("p (g c) -> p g c", g=G)
            for wl in range(WQ):
                pv = psmall.tile([128, 128], f32, tag="ps")
                nc.tensor.matmul(pv[:], xq[:, ds(wl * H, H)], wvT[:],
                                 start=True, stop=True)
                nc.vector.tensor_add(vtp4[:, wl, :, 0:16],
                                     pv[:].rearrange("p (g c) -> p g c", g=G),
                                     bvbc3)

            # B_s: k_g copied to q_g's partition rows (zero elsewhere); the
            # stationary side of the K=64 window matmul at base 64*(g//4)
            Bs = []
            for s_ in range(4):
                bt = hpool.tile([128, QCOLS], fp16, tag=f"b{s_}")
                nc.gpsimd.memset(bt[:], 0.0)
                for g in (s_, s_ + 4):
                    nc.gpsimd.dma_start(out=bt[g * 16:g * 16 + 8, :],
                                        in_=qkp[g * 16 + 8:g * 16 + 16, :])
                Bs.append(bt)
            return qkp, Bs, vtp4

        st = build(0)
        for q in range(NQ):
            nxt = build(q + 1) if q + 1 < NQ else None
            qkp, Bs, vtp4 = st

            # attention over this pass's w columns
            accq = apool.tile([128, OP * WQ], f32, tag="acc")
            accq3 = accq.rearrange("p (o wq) -> p o wq", wq=WQ)
            for wl in range(WQ):
                pq = pqk.tile([128, 1024], f32, tag="pq")
                for g in range(G):
                    b64 = 64 * (g // 4)
                    nc.tensor.matmul(
                        pq[:, ts(g, 128)],
                        Bs[g % 4][b64:b64 + 64, ds(wl * H, H)],
                        qkp[b64:b64 + 64, ds(wl * H, H)],
                        start=True, stop=True)
                et = epool.tile([128, 1024], bf16, tag="et")
                nc.scalar.activation(et[:, 0:512], pq[:, 0:512], Exp)
                nc.scalar.activation(et[:, 512:1024], pq[:, 512:1024], Exp)
                psv = psmall.tile([128, G * 17], f32, tag="ps")
                for g in range(G):
                    nc.tensor.matmul(psv[:, ds(g * 17, 17)],
                                     et[:, ts(g, 128)],
                                     vtp4[:, wl, g, :],
                                     start=True, stop=True)
                psv3 = psv.rearrange("p (g c) -> p g c", g=G)
                rc = rpool.tile([128, G], f32, tag="rc")
                nc.vector.reciprocal(rc[:], psv3[:, :, 16])
                rcb = bass.AP(tensor=rc[:].tensor, offset=rc[:].offset,
                              ap=[rc[:].ap[0], [1, G], [0, 16]])
                nc.vector.tensor_tensor(
                    accq3[:, :, wl].rearrange("p (g c) -> p g c", g=G),
                    psv3[:, :, 0:16], rcb, mybir.AluOpType.mult)
            nc.sync.dma_start(y_d.ap()[q], accq[:])
            st = nxt

    nc.compile()
    return nc


def _prep_host(x, w_qkv, qkv_gamma, qkv_beta, qkv_mean, qkv_var,
               sim_gamma, sim_beta, sim_mean, sim_var,
               out_gamma, out_beta, out_mean, out_var):
    bf16 = ml_dtypes.bfloat16
    x = np.asarray(x, np.float32)
    w_qkv = np.asarray(w_qkv, np.float32)

    qscale = np.asarray(qkv_gamma, np.float32) / np.sqrt(np.asarray(qkv_var, np.float32) + EPS)
    Wp = w_qkv * qscale[:, None]
    bp = np.asarray(qkv_beta, np.float32) - np.asarray(qkv_mean, np.float32) * qscale

    sscale = np.asarray(sim_gamma, np.float32) / np.sqrt(np.asarray(sim_var, np.float32) + EPS)
    outS = np.asarray(out_gamma, np.float32) / np.sqrt(np.asarray(out_var, np.float32) + EPS)
    outB = np.asarray(out_beta, np.float32) - np.asarray(out_mean, np.float32) * outS

    Wp_r = Wp.reshape(G, 32, C)
    bp_r = bp.reshape(G, 32)
    Wqk = np.zeros((128, C), np.float32)
    bqk = np.zeros(128, np.float32)
    Wv = np.zeros((128, C), np.float32)
    bv = np.zeros(128, np.float32)
    for g in range(G):
        Wqk[g * 16:g * 16 + 8] = Wp_r[g, 0:8]
        bqk[g * 16:g * 16 + 8] = bp_r[g, 0:8]
        Wqk[g * 16 + 8:g * 16 + 16] = Wp_r[g, 8:16] * sscale[g]
        bqk[g * 16 + 8:g * 16 + 16] = bp_r[g, 8:16] * sscale[g]
        oS = outS[g * 16:(g + 1) * 16]
        Wv[g * 16:(g + 1) * 16] = Wp_r[g, 16:32] * oS[:, None]
        bv[g * 16:(g + 1) * 16] = bp_r[g, 16:32] * oS + outB[g * 16:(g + 1) * 16]

    wqkT = np.ascontiguousarray(Wqk.T).astype(np.float16)
    wvT = np.ascontiguousarray(Wv.T).astype(np.float16)
    bqk2 = np.ascontiguousarray(bqk[:, None])
    bvbc = np.ascontiguousarray(np.broadcast_to(bv[None, :], (128, 128)))

    in_maps = []
    for n in range(NCORES):
        xp = np.ascontiguousarray(x[n].transpose(0, 2, 1).reshape(C, W * H)).astype(np.float16)
        in_maps.append({"xp": xp, "wqkT": wqkT, "wvT": wvT,
                        "bqk": bqk2, "bvbc": bvbc})
    return in_maps


def _get_nc():
    global _cached
    if _cached is None:
        _cached = _build_bass()
    return _cached


def run(inputs, trace=False):
    """Run on all 8 cores; returns (full_output, BassKernelResults)."""
    from concourse.bass_utils import run_bass_kernel_spmd
    nc = _get_nc()
    in_maps = _prep_host(**inputs)
    res = run_bass_kernel_spmd(nc, in_maps, list(range(NCORES)), trace=trace)
    out = np.empty((NCORES, OP, H, W), np.float32)
    for n in range(NCORES):
        yq = res.results[n]["y"]          # [NQ, H, OP, WQ]
        out[n] = yq.transpose(2, 1, 0, 3).reshape(OP, H, W)
    return out, res


def kernel(**inputs):
    out, _ = run(inputs, trace=False)
    return out


# revision 18
# speedup vs baseline: 1.1030x; 1.1030x over previous
"""AxialAttention (width=False, no positional) as a Bass/Tile kernel on 8 TRN2 NeuronCores.

Sharding: data-parallel over N (8 images -> 8 cores); conv/BN params replicated.

Math (per image, per w-column, per group g of 8; H=128, gp=16):
    qkv = BN1d(w_qkv @ x)            -> q (8ch), k (8ch), v (16ch) per group
    sim = softmax_j(BN(q.k))         (sim BN: additive part cancels in softmax;
                                      multiplicative part folded into k's weights)
    out = BN(sim @ v)                (output BN folded into v's weights/bias;
                                      softmax denominator divided at the end)

Device-side formulation per (w, g):
    qkT[j,i]  = sum_c k[c,j] q[c,i]          (TensorE, K=64 zero-padded window)
    eT[j,i]   = exp(qkT)                      (ScalarE, PSUM->SBUF bf16)
    usv[i,c], s[i] = eT.T @ [v | 1]           (TensorE; ones col gives softmax denom)
    y[op,h=i,w] = usv[i,c] / s[i]             (VectorE reciprocal + per-partition mult)

Layouts (per core):
    xp   [c, (w,h)]       host-pretransposed input, fp16
    qkp  [o_qk, (w,h)]    o_qk = g*16 + (q:0..8 | k:8..16); k rows pre-scaled by sim BN
    B_s  [o, (w,h)]       k_g copied to q_g's partition rows for g=s(mod 4), zeros
                          elsewhere (SBUF->SBUF partition-shift DMAs); stationary side
                          of the K=64 window matmul at partition base 64*(g//4)
    vtp  [j, (w, g, 16+1)] BN-folded v channels + interleaved ones columns, bf16
    acc  [h, (op, wq)]    per-pass output accumulator, fp32; y is returned as
                          [NQ, H, OP, WQ] and transposed to [OP, H, W] on host
"""

import numpy as np
import ml_dtypes

EPS = 1e-5
G = 8
C = 128
H = 128
W = 128
NCORES = 8
NQ = 4            # number of w-passes
WQ = W // NQ      # w-columns per pass
OP = 128          # out_planes

_cached = None


def _build_bass():
    from contextlib import ExitStack

    import concourse.bass as bass
    from concourse import bacc
    import concourse.tile as tile
    import concourse.mybir as mybir
    from concourse.bass import ts, ds

    f32, bf16 = mybir.dt.float32, mybir.dt.bfloat16
    fp16 = mybir.dt.float16
    Exp = mybir.ActivationFunctionType.Exp

    nc = bacc.Bacc("TRN2", target_bir_lowering=False, debug=False)
    xp_d = nc.dram_tensor("xp", [C, W * H], fp16, kind="ExternalInput")
    wqkT_d = nc.dram_tensor("wqkT", [C, 128], fp16, kind="ExternalInput")
    wvT_d = nc.dram_tensor("wvT", [C, 128], fp16, kind="ExternalInput")
    bqk_d = nc.dram_tensor("bqk", [128, 1], f32, kind="ExternalInput")
    bvbc_d = nc.dram_tensor("bvbc", [128, 128], f32, kind="ExternalInput")
    y_d = nc.dram_tensor("y", [NQ, H, OP, WQ], f32, kind="ExternalOutput")

    QCOLS = WQ * H           # free columns per pass in (w,h) layout

    with tile.TileContext(nc) as tc, ExitStack() as ctx:
        const = ctx.enter_context(tc.tile_pool(name="const", bufs=1))
        wqkT = const.tile([C, 128], fp16)
        wvT = const.tile([C, 128], fp16)
        bqk = const.tile([128, 1], f32)
        bvbc = const.tile([128, 128], f32)
        nc.sync.dma_start(wqkT[:], wqkT_d.ap())
        nc.sync.dma_start(wvT[:], wvT_d.ap())
        nc.sync.dma_start(bqk[:], bqk_d.ap())
        nc.sync.dma_start(bvbc[:], bvbc_d.ap())

        apool = ctx.enter_context(tc.tile_pool(name="apool", bufs=2))
        xpool = ctx.enter_context(tc.tile_pool(name="xpool", bufs=2))
        hpool = ctx.enter_context(tc.tile_pool(name="hpool", bufs=2))
        epool = ctx.enter_context(tc.tile_pool(name="epool", bufs=4))
        rpool = ctx.enter_context(tc.tile_pool(name="rpool", bufs=4))
        psmall = ctx.enter_context(tc.tile_pool(name="psmall", bufs=2, space="PSUM"))
        pqk = ctx.enter_context(tc.tile_pool(name="pqk", bufs=3, space="PSUM"))

        NP = WQ // 2

        def build(q):
            """Emit the load/projection/pair-build stage for pass q."""
            xq = xpool.tile([C, QCOLS], fp16, tag="xq")
            qtr = QCOLS // 4
            for i in range(4):
                nc.sync.dma_start(xq[:, ds(i * qtr, qtr)],
                                  xp_d.ap()[:, ds(q * QCOLS + i * qtr, qtr)])

            # q/k projection: qkp[o,(w,h)] = wqkT.T @ xq + bqk (k rows sim-scaled)
            qkp = hpool.tile([128, QCOLS], fp16, tag="qkp")
            for i in range(QCOLS // 512):
                pt = pqk.tile([128, 512], f32, tag="pq")
                nc.tensor.matmul(pt[:], wqkT[:], xq[:, ts(i, 512)],
                                 start=True, stop=True)
                nc.vector.tensor_scalar_add(qkp[:, ts(i, 512)], pt[:], bqk[:, 0:1])

            # vT projection: vtp[j=h, (w, g, c|1)] = xq_w.T @ wvT + bv (BN-folded)
            vtp = hpool.tile([128, WQ * G * 17], bf16, tag="vtp")
            vtp4 = vtp.rearrange("p (w g c) -> p w g c", w=WQ, g=G)
            nc.vector.memset(vtp4[:, :, :, 16:17], 1.0)
            bvbc3 = bvbc.rearrange("p (g c) -> p g c", g=G)
            for wl in range(WQ):
                pv = psmall.tile([128, 128], f32, tag="ps")
                nc.tensor.matmul(pv[:], xq[:, ds(wl * H, H)], wvT[:],
                                 start=True, stop=True)
                nc.vector.tensor_add(vtp4[:, wl, :, 0:16],
                                     pv[:].rearrange("p (g c) -> p g c", g=G),
                                     bvbc3)

            # B_s: k_g copied to q_g's partition rows (zero elsewhere); the
            # stationary side of the K=64 window matmul at base 64*(g//4)
            Bs = []
            for s_ in range(4):
                bt = hpool.tile([128, QCOLS], fp16, tag=f"b{s_}")
                nc.gpsimd.memset(bt[:], 0.0)
                for g in (s_, s_ + 4):
                    nc.gpsimd.dma_start(out=bt[g * 16:g * 16 + 8, :],
                                        in_=qkp[g * 16 + 8:g * 16 + 16, :])
                Bs.append(bt)
            return qkp, Bs, vtp4

        st = build(0)
        for q in range(NQ):
            nxt = build(q + 1) if q + 1 < NQ else None
            qkp, Bs, vtp4 = st

            # attention over this pass's w columns
            accq = apool.tile([128, OP * WQ], f32, tag="acc")
            accq3 = accq.rearrange("p (o wq) -> p o wq", wq=WQ)
            for wl in range(WQ):
                pq = pqk.tile([128, 1024], f32, tag="pq")
                for g in range(G):
                    b64 = 64 * (g // 4)
                    nc.tensor.matmul(
                        pq[:, ts(g, 128)],
                        Bs[g % 4][b64:b64 + 64, ds(wl * H, H)],
                        qkp[b64:b64 + 64, ds(wl * H, H)],
                        start=True, stop=True)
                et = epool.tile([128, 1024], bf16, tag="et")
                nc.scalar.activation(et[:], pq[:], Exp)
                psv = psmall.tile([128, G * 17], f32, tag="ps")
                for g in range(G):
                    nc.tensor.matmul(psv[:, ds(g * 17, 17)],
                                     et[:, ts(g, 128)],
                                     vtp4[:, wl, g, :],
                                     start=True, stop=True)
                psv3 = psv.rearrange("p (g c) -> p g c", g=G)
                rc = rpool.tile([128, G], f32, tag="rc")
                nc.vector.reciprocal(rc[:], psv3[:, :, 16])
                rcb = bass.AP(tensor=rc[:].tensor, offset=rc[:].offset,
                              ap=[rc[:].ap[0], [1, G], [0, 16]])
                nc.vector.tensor_tensor(
                    accq3[:, :, wl].rearrange("p (g c) -> p g c", g=G),
                    psv3[:, :, 0:16], rcb, mybir.AluOpType.mult)
            nc.sync.dma_start(y_d.ap()[q], accq[:])
            st = nxt

    nc.compile()
    return nc


def _prep_host(x, w_qkv, qkv_gamma, qkv_beta, qkv_mean, qkv_var,
               sim_gamma, sim_beta, sim_mean, sim_var,
               out_gamma, out_beta, out_mean, out_var):
    bf16 = ml_dtypes.bfloat16
    x = np.asarray(x, np.float32)
    w_qkv = np.asarray(w_qkv, np.float32)

    qscale = np.asarray(qkv_gamma, np.float32) / np.sqrt(np.asarray(qkv_var, np.float32) + EPS)
    Wp = w_qkv * qscale[:, None]
    bp = np.asarray(qkv_beta, np.float32) - np.asarray(qkv_mean, np.float32) * qscale

    sscale = np.asarray(sim_gamma, np.float32) / np.sqrt(np.asarray(sim_var, np.float32) + EPS)
    outS = np.asarray(out_gamma, np.float32) / np.sqrt(np.asarray(out_var, np.float32) + EPS)
    outB = np.asarray(out_beta, np.float32) - np.asarray(out_mean, np.float32) * outS

    Wp_r = Wp.reshape(G, 32, C)
    bp_r = bp.reshape(G, 32)
    Wqk = np.zeros((128, C), np.float32)
    bqk = np.zeros(128, np.float32)
    Wv = np.zeros((128, C), np.float32)
    bv = np.zeros(128, np.float32)
    for g in range(G):
        Wqk[g * 16:g * 16 + 8] = Wp_r[g, 0:8]
        bqk[g * 16:g * 16 + 8] = bp_r[g, 0:8]
        Wqk[g * 16 + 8:g * 16 + 16] = Wp_r[g, 8:16] * sscale[g]
        bqk[g * 16 + 8:g * 16 + 16] = bp_r[g, 8:16] * sscale[g]
        oS = outS[g * 16:(g + 1) * 16]
        Wv[g * 16:(g + 1) * 16] = Wp_r[g, 16:32] * oS[:, None]
        bv[g * 16:(g + 1) * 16] = bp_r[g, 16:32] * oS + outB[g * 16:(g + 1) * 16]

    wqkT = np.ascontiguousarray(Wqk.T).astype(np.float16)
    wvT = np.ascontiguousarray(Wv.T).astype(np.float16)
    bqk2 = np.ascontiguousarray(bqk[:, None])
    bvbc = np.ascontiguousarray(np.broadcast_to(bv[None, :], (128, 128)))

    in_maps = []
    for n in range(NCORES):
        xp = np.ascontiguousarray(x[n].transpose(0, 2, 1).reshape(C, W * H)).astype(np.float16)
        in_maps.append({"xp": xp, "wqkT": wqkT, "wvT": wvT,
                        "bqk": bqk2, "bvbc": bvbc})
    return in_maps


def _get_nc():
    global _cached
    if _cached is None:
        _cached = _build_bass()
    return _cached


def run(inputs, trace=False):
    """Run on all 8 cores; returns (full_output, BassKernelResults)."""
    from concourse.bass_utils import run_bass_kernel_spmd
    nc = _get_nc()
    in_maps = _prep_host(**inputs)
    res = run_bass_kernel_spmd(nc, in_maps, list(range(NCORES)), trace=trace)
    out = np.empty((NCORES, OP, H, W), np.float32)
    for n in range(NCORES):
        yq = res.results[n]["y"]          # [NQ, H, OP, WQ]
        out[n] = yq.transpose(2, 1, 0, 3).reshape(OP, H, W)
    return out, res


def kernel(**inputs):
    out, _ = run(inputs, trace=False)
    return out


# revision 19
# speedup vs baseline: 1.1302x; 1.0247x over previous
"""AxialAttention (width=False, no positional) as a Bass/Tile kernel on 8 TRN2 NeuronCores.

Sharding: data-parallel over N (8 images -> 8 cores); conv/BN params replicated.

Math (per image, per w-column, per group g of 8; H=128, gp=16):
    qkv = BN1d(w_qkv @ x)            -> q (8ch), k (8ch), v (16ch) per group
    sim = softmax_j(BN(q.k))         (sim BN: additive part cancels in softmax;
                                      multiplicative part folded into k's weights)
    out = BN(sim @ v)                (output BN folded into v's weights/bias;
                                      softmax denominator divided at the end)

Device-side formulation per (w, g):
    qkT[j,i]  = sum_c k[c,j] q[c,i]          (TensorE, K=64 zero-padded window)
    eT[j,i]   = exp(qkT)                      (ScalarE, PSUM->SBUF bf16)
    usv[i,c], s[i] = eT.T @ [v | 1]           (TensorE; ones col gives softmax denom)
    y[op,h=i,w] = usv[i,c] / s[i]             (VectorE reciprocal + per-partition mult)

Layouts (per core):
    xp   [c, (w,h)]       host-pretransposed input, fp16
    qkp  [o_qk, (w,h)]    o_qk = g*16 + (q:0..8 | k:8..16); k rows pre-scaled by sim BN
    B_s  [o, (w,h)]       k_g copied to q_g's partition rows for g=s(mod 4), zeros
                          elsewhere (SBUF->SBUF partition-shift DMAs); stationary side
                          of the K=64 window matmul at partition base 64*(g//4)
    vtp  [j, (w, g, 16+1)] BN-folded v channels + interleaved ones columns, bf16
    acc  [h, (op, wq)]    per-pass output accumulator, fp32; y is returned as
                          [NQ, H, OP, WQ] and transposed to [OP, H, W] on host
"""

import numpy as np
import ml_dtypes

EPS = 1e-5
G = 8
C = 128
H = 128
W = 128
NCORES = 8
NQ = 4            # number of w-passes
WQ = W // NQ      # w-columns per pass
OP = 128          # out_planes

_cached = None


def _build_bass():
    from contextlib import ExitStack

    import concourse.bass as bass
    from concourse import bacc
    import concourse.tile as tile
    import concourse.mybir as mybir
    from concourse.bass import ts, ds

    f32, bf16 = mybir.dt.float32, mybir.dt.bfloat16
    fp16 = mybir.dt.float16
    Exp = mybir.ActivationFunctionType.Exp

    nc = bacc.Bacc("TRN2", target_bir_lowering=False, debug=False)
    xp_d = nc.dram_tensor("xp", [C, W * H], fp16, kind="ExternalInput")
    wqkT_d = nc.dram_tensor("wqkT", [C, 128], fp16, kind="ExternalInput")
    wvT_d = nc.dram_tensor("wvT", [C, 128], fp16, kind="ExternalInput")
    bqk_d = nc.dram_tensor("bqk", [128, 1], f32, kind="ExternalInput")
    bvbc_d = nc.dram_tensor("bvbc", [128, 128], f32, kind="ExternalInput")
    y_d = nc.dram_tensor("y", [NQ * 4, H, OP, 8], f32, kind="ExternalOutput")

    QCOLS = WQ * H           # free columns per pass in (w,h) layout

    with tile.TileContext(nc) as tc, ExitStack() as ctx:
        const = ctx.enter_context(tc.tile_pool(name="const", bufs=1))
        wqkT = const.tile([C, 128], fp16)
        wvT = const.tile([C, 128], fp16)
        bqk = const.tile([128, 1], f32)
        bvbc = const.tile([128, 128], f32)
        nc.sync.dma_start(wqkT[:], wqkT_d.ap())
        nc.sync.dma_start(bqk[:], bqk_d.ap())

        apool = ctx.enter_context(tc.tile_pool(name="apool", bufs=3))
        xpool = ctx.enter_context(tc.tile_pool(name="xpool", bufs=2))
        hpool = ctx.enter_context(tc.tile_pool(name="hpool", bufs=2))
        epool = ctx.enter_context(tc.tile_pool(name="epool", bufs=4))
        rpool = ctx.enter_context(tc.tile_pool(name="rpool", bufs=4))
        psmall = ctx.enter_context(tc.tile_pool(name="psmall", bufs=2, space="PSUM"))
        pqk = ctx.enter_context(tc.tile_pool(name="pqk", bufs=3, space="PSUM"))

        NP = WQ // 2

        def build(q):
            """Emit the load/projection/pair-build stage for pass q."""
            xq = xpool.tile([C, QCOLS], fp16, tag="xq")
            qtr = QCOLS // 4
            for i in range(4):
                nc.sync.dma_start(xq[:, ds(i * qtr, qtr)],
                                  xp_d.ap()[:, ds(q * QCOLS + i * qtr, qtr)])
            if q == 0:
                nc.sync.dma_start(wvT[:], wvT_d.ap())
                nc.sync.dma_start(bvbc[:], bvbc_d.ap())

            # q/k projection: qkp[o,(w,h)] = wqkT.T @ xq + bqk (k rows sim-scaled)
            qkp = hpool.tile([128, QCOLS], fp16, tag="qkp")
            for i in range(QCOLS // 512):
                pt = pqk.tile([128, 512], f32, tag="pq")
                nc.tensor.matmul(pt[:], wqkT[:], xq[:, ts(i, 512)],
                                 start=True, stop=True)
                nc.vector.tensor_scalar_add(qkp[:, ts(i, 512)], pt[:], bqk[:, 0:1])

            # vT projection: vtp[j=h, (w, g, c|1)] = xq_w.T @ wvT + bv (BN-folded)
            vtp = hpool.tile([128, WQ * G * 17], bf16, tag="vtp")
            vtp4 = vtp.rearrange("p (w g c) -> p w g c", w=WQ, g=G)
            nc.vector.memset(vtp4[:, :, :, 16:17], 1.0)
            bvbc3 = bvbc.rearrange("p (g c) -> p g c", g=G)
            for wl in range(WQ):
                pv = psmall.tile([128, 128], f32, tag="ps")
                nc.tensor.matmul(pv[:], xq[:, ds(wl * H, H)], wvT[:],
                                 start=True, stop=True)
                nc.vector.tensor_add(vtp4[:, wl, :, 0:16],
                                     pv[:].rearrange("p (g c) -> p g c", g=G),
                                     bvbc3)

            # B_s: k_g copied to q_g's partition rows (zero elsewhere); the
            # stationary side of the K=64 window matmul at base 64*(g//4)
            Bs = []
            for s_ in range(4):
                bt = hpool.tile([128, QCOLS], fp16, tag=f"b{s_}")
                nc.gpsimd.memset(bt[:], 0.0)
                for g in (s_, s_ + 4):
                    nc.gpsimd.dma_start(out=bt[g * 16:g * 16 + 8, :],
                                        in_=qkp[g * 16 + 8:g * 16 + 16, :])
                Bs.append(bt)
            return qkp, Bs, vtp4

        st = build(0)
        for q in range(NQ):
            nxt = build(q + 1) if q + 1 < NQ else None
            qkp, Bs, vtp4 = st

            # attention over this pass's w columns
            accq = None
            for wl in range(WQ):
                if wl % 8 == 0:
                    accq = apool.tile([128, OP * 8], f32, tag="acc")
                    accq3 = accq.rearrange("p (o wq) -> p o wq", wq=8)
                pq = pqk.tile([128, 1024], f32, tag="pq")
                for g in range(G):
                    b64 = 64 * (g // 4)
                    nc.tensor.matmul(
                        pq[:, ts(g, 128)],
                        Bs[g % 4][b64:b64 + 64, ds(wl * H, H)],
                        qkp[b64:b64 + 64, ds(wl * H, H)],
                        start=True, stop=True)
                et = epool.tile([128, 1024], bf16, tag="et")
                nc.scalar.activation(et[:], pq[:], Exp)
                psv = psmall.tile([128, G * 17], f32, tag="ps")
                for g in range(G):
                    nc.tensor.matmul(psv[:, ds(g * 17, 17)],
                                     et[:, ts(g, 128)],
                                     vtp4[:, wl, g, :],
                                     start=True, stop=True)
                psv3 = psv.rearrange("p (g c) -> p g c", g=G)
                rc = rpool.tile([128, G], f32, tag="rc")
                nc.vector.reciprocal(rc[:], psv3[:, :, 16])
                rcb = bass.AP(tensor=rc[:].tensor, offset=rc[:].offset,
                              ap=[rc[:].ap[0], [1, G], [0, 16]])
                nc.vector.tensor_tensor(
                    accq3[:, :, wl % 8].rearrange("p (g c) -> p g c", g=G),
                    psv3[:, :, 0:16], rcb, mybir.AluOpType.mult)
                if wl % 8 == 7:
                    nc.sync.dma_start(y_d.ap()[q * 4 + wl // 8], accq[:])
            st = nxt

    nc.compile()
    return nc


def _prep_host(x, w_qkv, qkv_gamma, qkv_beta, qkv_mean, qkv_var,
               sim_gamma, sim_beta, sim_mean, sim_var,
               out_gamma, out_beta, out_mean, out_var):
    bf16 = ml_dtypes.bfloat16
    x = np.asarray(x, np.float32)
    w_qkv = np.asarray(w_qkv, np.float32)

    qscale = np.asarray(qkv_gamma, np.float32) / np.sqrt(np.asarray(qkv_var, np.float32) + EPS)
    Wp = w_qkv * qscale[:, None]
    bp = np.asarray(qkv_beta, np.float32) - np.asarray(qkv_mean, np.float32) * qscale

    sscale = np.asarray(sim_gamma, np.float32) / np.sqrt(np.asarray(sim_var, np.float32) + EPS)
    outS = np.asarray(out_gamma, np.float32) / np.sqrt(np.asarray(out_var, np.float32) + EPS)
    outB = np.asarray(out_beta, np.float32) - np.asarray(out_mean, np.float32) * outS

    Wp_r = Wp.reshape(G, 32, C)
    bp_r = bp.reshape(G, 32)
    Wqk = np.zeros((128, C), np.float32)
    bqk = np.zeros(128, np.float32)
    Wv = np.zeros((128, C), np.float32)
    bv = np.zeros(128, np.float32)
    for g in range(G):
        Wqk[g * 16:g * 16 + 8] = Wp_r[g, 0:8]
        bqk[g * 16:g * 16 + 8] = bp_r[g, 0:8]
        Wqk[g * 16 + 8:g * 16 + 16] = Wp_r[g, 8:16] * sscale[g]
        bqk[g * 16 + 8:g * 16 + 16] = bp_r[g, 8:16] * sscale[g]
        oS = outS[g * 16:(g + 1) * 16]
        Wv[g * 16:(g + 1) * 16] = Wp_r[g, 16:32] * oS[:, None]
        bv[g * 16:(g + 1) * 16] = bp_r[g, 16:32] * oS + outB[g * 16:(g + 1) * 16]

    wqkT = np.ascontiguousarray(Wqk.T).astype(np.float16)
    wvT = np.ascontiguousarray(Wv.T).astype(np.float16)
    bqk2 = np.ascontiguousarray(bqk[:, None])
    bvbc = np.ascontiguousarray(np.broadcast_to(bv[None, :], (128, 128)))

    in_maps = []
    for n in range(NCORES):
        xp = np.ascontiguousarray(x[n].transpose(0, 2, 1).reshape(C, W * H)).astype(np.float16)
        in_maps.append({"xp": xp, "wqkT": wqkT, "wvT": wvT,
                        "bqk": bqk2, "bvbc": bvbc})
    return in_maps


def _get_nc():
    global _cached
    if _cached is None:
        _cached = _build_bass()
    return _cached


def run(inputs, trace=False):
    """Run on all 8 cores; returns (full_output, BassKernelResults)."""
    from concourse.bass_utils import run_bass_kernel_spmd
    nc = _get_nc()
    in_maps = _prep_host(**inputs)
    res = run_bass_kernel_spmd(nc, in_maps, list(range(NCORES)), trace=trace)
    out = np.empty((NCORES, OP, H, W), np.float32)
    for n in range(NCORES):
        yq = res.results[n]["y"]          # [NQ*4, H, OP, 8]
        out[n] = yq.transpose(2, 1, 0, 3).reshape(OP, H, W)
    return out, res


def kernel(**inputs):
    out, _ = run(inputs, trace=False)
    return out
